# revision 2
# baseline (speedup 1.0000x reference)
"""Viterbi CRF decode kernel for Trainium2 (8 NeuronCores, SPMD data-parallel).

Problem: inputs [512, 2048, 32] f32 potentials, transitions [32, 32] f32.
Output: one_hot(viterbi_tags) [512, 2048, 32] f32.

Sharding: batch 512 -> 8 cores x 64. Per core:
  Forward scan over T (exact fp32, same op order as reference):
    v[b, j, i]   = state[b, i] + T[i, j]          (broadcast TT add, j-major)
    maxs[b, j]   = max_i v                        (strided X-reduce)
    state'[b, j] = maxs + pot[b, t, j]
    bp via first-argmax trick: mask = (v == maxs), sel = mask * (32 - i),
    bpenc[b, j] = max_i sel  ->  int8 store  (bp = 32 - bpenc, ties -> first i,
    matching jnp.argmax; comparisons are exact since v/maxs are exact).
  Backward: tag one-hots generated directly into the output buffer:
    enc_t = max_j(bpenc_t * h_{t+1})  (h one-hot), h_t = (iotaenc == enc_t).
"""

import numpy as np

B, T, K = 512, 2048, 32
NCORES = 8
BS = B // NCORES  # 64 batches per core
CHUNK = 128       # time chunk for potentials / output DMA

_cached = {}


def _split_waits(nc, mybir):
    """This walrus build accepts at most one sync-wait per instruction; move
    extra waits onto preceding EventSemaphore instructions on the same engine."""
    for f in nc.m.functions:
        for blk in f.blocks:
            new_list = []
            for ins in blk.instructions:
                si = ins.sync_info
                if si is not None and len(si.on_wait) > 1:
                    waits = list(si.on_wait)
                    keep, moved = waits[-1:], waits[:-1]
                    for ci, w in enumerate(moved):
                        nop = mybir.InstEventSemaphore(name=f"{ins.name}-ws{ci}")
                        nop.engine = ins.engine
                        nop.sync_info = mybir.SyncInfo(on_wait=[w], on_update=[])
                        new_list.append(nop)
                    ins.sync_info = mybir.SyncInfo(
                        on_wait=keep, on_update=list(si.on_update)
                    )
                new_list.append(ins)
            blk.instructions = new_list


def _build():
    import concourse.bass as bass
    import concourse.mybir as mybir
    from concourse.tile import TileContext

    f32 = mybir.dt.float32
    i8 = mybir.dt.int8
    ADD = mybir.AluOpType.add
    MUL = mybir.AluOpType.mult
    MAX = mybir.AluOpType.max
    EQ = mybir.AluOpType.is_equal
    X = mybir.AxisListType.X

    nc = bass.Bass("TRN2", target_bir_lowering=False, num_devices=NCORES)
    pot = nc.declare_dram_parameter("pot", [BS, T, K], f32, isOutput=False)
    transrep = nc.declare_dram_parameter("transrep", [BS, K * K], f32, isOutput=False)
    iotaenc = nc.declare_dram_parameter("iotaenc", [BS, K], f32, isOutput=False)
    out = nc.declare_dram_parameter("out", [BS, T, K], f32, isOutput=True)

    with TileContext(nc) as tc:
        with (
            tc.tile_pool(name="const", bufs=1) as cpool,
            tc.tile_pool(name="bps", bufs=1) as bpool,
        ):
            tr = cpool.tile([BS, K * K], f32)
            nc.sync.dma_start(out=tr[:], in_=transrep[:])
            io = cpool.tile([BS, K], f32)
            nc.sync.dma_start(out=io[:], in_=iotaenc[:])
            statefin = cpool.tile([BS, K], f32)
            bps = bpool.tile([BS, (T - 1) * K], i8)

            tr3d = tr[:].rearrange("p (j i) -> p j i", j=K)
            ioBC = io[:].rearrange("p (a i) -> p a i", a=1).broadcast_to([BS, K, K])

            # ---------------- forward ----------------
            with (
                tc.tile_pool(name="pot", bufs=2) as ppool,
                tc.tile_pool(name="fwd", bufs=3) as fpool,
                tc.tile_pool(name="st", bufs=3) as spool,
            ):
                state = None
                for c in range(T // CHUNK):
                    ptile = ppool.tile([BS, CHUNK, K], f32, tag="pot")
                    nc.sync.dma_start(
                        out=ptile[:], in_=pot[:, c * CHUNK : (c + 1) * CHUNK, :]
                    )
                    for s in range(CHUNK):
                        t = c * CHUNK + s
                        if t == 0:
                            state = spool.tile([BS, K], f32, tag="state")
                            nc.vector.tensor_copy(state[:], ptile[:, 0, :])
                            continue
                        v = fpool.tile([BS, K * K], f32, tag="v")
                        v3d = v[:].rearrange("p (j i) -> p j i", j=K)
                        sbc = (
                            state[:]
                            .rearrange("p (a i) -> p a i", a=1)
                            .broadcast_to([BS, K, K])
                        )
                        nc.vector.tensor_tensor(out=v3d, in0=sbc, in1=tr3d, op=ADD)
                        maxs = fpool.tile([BS, K], f32, tag="maxs")
                        nc.vector.tensor_reduce(out=maxs[:], in_=v3d, axis=X, op=MAX)
                        newstate = spool.tile([BS, K], f32, tag="state")
                        nc.vector.tensor_tensor(
                            out=newstate[:], in0=maxs[:], in1=ptile[:, s, :], op=ADD
                        )
                        mask = fpool.tile([BS, K * K], f32, tag="mask")
                        mask3d = mask[:].rearrange("p (j i) -> p j i", j=K)
                        maxsBC = (
                            maxs[:]
                            .rearrange("p (j a) -> p j a", a=1)
                            .broadcast_to([BS, K, K])
                        )
                        nc.vector.tensor_tensor(out=mask3d, in0=v3d, in1=maxsBC, op=EQ)
                        sel = fpool.tile([BS, K * K], f32, tag="sel")
                        sel3d = sel[:].rearrange("p (j i) -> p j i", j=K)
                        nc.vector.tensor_tensor(out=sel3d, in0=mask3d, in1=ioBC, op=MUL)
                        bslot = bps[:, (t - 1) * K : t * K]
                        nc.vector.tensor_reduce(out=bslot, in_=sel3d, axis=X, op=MAX)
                        state = newstate
                    del ptile
                nc.vector.tensor_copy(statefin[:], state[:])

            # ---------------- backward ----------------
            with (
                tc.tile_pool(name="outp", bufs=2) as opool,
                tc.tile_pool(name="bwd", bufs=4) as wpool,
            ):
                mfin = wpool.tile([BS, 1], f32, tag="enc")
                nc.vector.tensor_reduce(out=mfin[:], in_=statefin[:], axis=X, op=MAX)
                maskf = wpool.tile([BS, K], f32, tag="prod")
                nc.vector.tensor_scalar(
                    out=maskf[:], in0=statefin[:], scalar1=mfin[:], scalar2=None, op0=EQ
                )
                self_ = wpool.tile([BS, K], f32, tag="prod")
                nc.vector.tensor_tensor(out=self_[:], in0=maskf[:], in1=io[:], op=MUL)
                enc = wpool.tile([BS, 1], f32, tag="enc")
                nc.vector.tensor_reduce(out=enc[:], in_=self_[:], axis=X, op=MAX)

                ochunk = opool.tile([BS, CHUNK, K], f32, tag="out")
                nc.vector.tensor_scalar(
                    out=ochunk[:, CHUNK - 1, :], in0=io[:], scalar1=enc[:],
                    scalar2=None, op0=EQ,
                )
                hprev = ochunk[:, CHUNK - 1, :]
                for t in range(T - 2, -1, -1):
                    s = t % CHUNK
                    prod = wpool.tile([BS, K], f32, tag="prod")
                    nc.vector.tensor_tensor(
                        out=prod[:], in0=bps[:, t * K : (t + 1) * K], in1=hprev, op=MUL
                    )
                    enc2 = wpool.tile([BS, 1], f32, tag="enc")
                    nc.vector.tensor_reduce(out=enc2[:], in_=prod[:], axis=X, op=MAX)
                    nc.vector.tensor_scalar(
                        out=ochunk[:, s, :], in0=io[:], scalar1=enc2[:],
                        scalar2=None, op0=EQ,
                    )
                    hprev = ochunk[:, s, :]
                    if s == 0:
                        c = t // CHUNK
                        nc.sync.dma_start(
                            out=out[:, c * CHUNK : (c + 1) * CHUNK, :], in_=ochunk[:]
                        )
                        if t > 0:
                            ochunk = opool.tile([BS, CHUNK, K], f32, tag="out")

    _split_waits(nc, mybir)
    return nc


def _get_program():
    if "nc" not in _cached:
        _cached["nc"] = _build()
    return _cached["nc"]


def _ensure_profile_hook():
    import sys
    import types

    try:
        from antenv.axon_hooks import get_axon_ntff_profile_hook  # noqa: F401
        return
    except ImportError:
        pass
    import antenv
    from trn_agent_boot.trn_boot import _ntff_profile_via_ctypes

    m = types.ModuleType("antenv.axon_hooks")
    holder = {"h": None}
    m.set_axon_ntff_profile_hook = lambda h: holder.__setitem__("h", h)
    m.get_axon_ntff_profile_hook = lambda: holder["h"]
    sys.modules["antenv.axon_hooks"] = m
    antenv.axon_hooks = m
    m.set_axon_ntff_profile_hook(
        _ntff_profile_via_ctypes("/opt/axon/libaxon_pjrt.so")
    )


def _run(inputs, transitions, trace=False):
    from concourse.bass_utils import run_bass_kernel_spmd

    if trace:
        _ensure_profile_hook()

    nc = _get_program()
    transrep = np.broadcast_to(
        np.ascontiguousarray(transitions.T).reshape(1, K * K), (BS, K * K)
    ).copy()
    iota = np.broadcast_to(
        (K - np.arange(K, dtype=np.float32)).reshape(1, K), (BS, K)
    ).copy()
    in_maps = [
        {
            "pot": np.ascontiguousarray(inputs[c * BS : (c + 1) * BS]),
            "transrep": transrep,
            "iotaenc": iota,
        }
        for c in range(NCORES)
    ]
    res = run_bass_kernel_spmd(nc, in_maps, list(range(NCORES)), trace=trace)
    full = np.concatenate([res.results[c]["out"] for c in range(NCORES)], axis=0)
    return full, res


def kernel(inputs, transitions):
    inputs = np.asarray(inputs, dtype=np.float32)
    transitions = np.asarray(transitions, dtype=np.float32)
    full, _ = _run(inputs, transitions, trace=False)
    return full.astype(np.float32)


# revision 3
# speedup vs baseline: 1.1477x; 1.1477x over previous
"""Viterbi CRF decode kernel for Trainium2 (8 NeuronCores, SPMD data-parallel).

Problem: inputs [512, 2048, 32] f32 potentials, transitions [32, 32] f32.
Output: one_hot(viterbi_tags) [512, 2048, 32] f32.

Sharding: batch 512 -> 8 cores x 64 (partition dim = batch). Per core:

Forward scan (exact fp32, same op order as the reference):
  v[b, j, i]   = state[b, i] + T[i, j]      (broadcast TT add, j-major layout)
  maxs[b, j]   = max_i v                    (strided X-reduce)
  state'[b, j] = maxs + pot[b, t, j]        (on GPSIMD, off the DVE queue)
  backpointers via a fused custom DVE op:
    sel = (v == maxs_bcast) ? (1024 - Idx) : 0     [CRF_SELECT_EQ_IDX]
    w[b, j] = max_i sel  -> int16            (w = 1024 - 32j - argmax_i,
    decreasing-index encoding makes ties resolve to the FIRST i, matching
    jnp.argmax; v/maxs comparisons are exact fp32)

Backward pass emits one-hot rows directly into the output buffer:
  wv = max_j(w_t[j] * h_{t+1}[j]); e = 1024 - 32*x_{t+1};
  h_t[k] = (k + wv == e); x_t = e - wv.
"""

import numpy as np

B, T, K = 512, 2048, 32
NCORES = 8
BS = B // NCORES  # 64 batches per core
CHUNK = 64        # time chunk for potentials / output DMA

_cached = {}

_OP_SHAS = {"v3": "130386e9d5d4f156", "v4": "874d42a2619fbd85"}


def _register_custom_op():
    import concourse.dve_ops as dve_ops
    from concourse.dve_ops import DveOp, OPS
    from concourse.dve_spec import Spec, Src0, Src1, C0, Idx, Zero, select, eq

    for o in OPS:
        if o.name == "CRF_SELECT_EQ_IDX":
            return o

    def _ref(in0, in1, s0, s1, imm2):
        n = in0.shape[-1] if in0.ndim == 2 else in0.shape[-1] * in0.shape[-2]
        idx = np.arange(n, dtype=np.float32).reshape(in0.shape[1:])
        return np.where(in0 == in1, s0 - idx[None], 0.0)

    spec = Spec(body=select(eq(Src0, Src1), C0 - Idx, Zero), reference=_ref)
    op = DveOp("CRF_SELECT_EQ_IDX", spec, subdim=False, uops_sha=dict(_OP_SHAS))
    OPS.append(op)
    dve_ops._SUB_OPCODE_FOR_NAME[op.name] = (
        dve_ops._CUSTOM_DVE_ROW_BASE + len(OPS) - 1
    )
    return op


def _split_waits(nc, mybir):
    """This walrus build accepts at most one sync-wait per instruction; move
    extra waits onto preceding EventSemaphore instructions on the same engine."""
    for f in nc.m.functions:
        for blk in f.blocks:
            new_list = []
            for ins in blk.instructions:
                si = ins.sync_info
                if si is not None and len(si.on_wait) > 1:
                    waits = list(si.on_wait)
                    keep, moved = waits[-1:], waits[:-1]
                    for ci, w in enumerate(moved):
                        nop = mybir.InstEventSemaphore(name=f"{ins.name}-ws{ci}")
                        nop.engine = ins.engine
                        nop.sync_info = mybir.SyncInfo(on_wait=[w], on_update=[])
                        new_list.append(nop)
                    ins.sync_info = mybir.SyncInfo(
                        on_wait=keep, on_update=list(si.on_update)
                    )
                new_list.append(ins)
            blk.instructions = new_list


def _build():
    import concourse.bass as bass
    import concourse.mybir as mybir
    from concourse.tile import TileContext

    OP = _register_custom_op()

    f32 = mybir.dt.float32
    i16 = mybir.dt.int16
    ADD = mybir.AluOpType.add
    SUB = mybir.AluOpType.subtract
    MUL = mybir.AluOpType.mult
    MAX = mybir.AluOpType.max
    EQ = mybir.AluOpType.is_equal
    X = mybir.AxisListType.X

    nc = bass.Bass("TRN2", target_bir_lowering=False, num_devices=NCORES)
    pot = nc.declare_dram_parameter("pot", [BS, T, K], f32, isOutput=False)
    transrep = nc.declare_dram_parameter("transrep", [BS, K * K], f32, isOutput=False)
    iota = nc.declare_dram_parameter("iota", [BS, K], f32, isOutput=False)
    out = nc.declare_dram_parameter("out", [BS, T, K], f32, isOutput=True)

    with TileContext(nc) as tc:
        with (
            tc.tile_pool(name="const", bufs=1) as cpool,
            tc.tile_pool(name="bps", bufs=1) as bpool,
        ):
            tr = cpool.tile([BS, K * K], f32)
            nc.sync.dma_start(out=tr[:], in_=transrep[:])
            io = cpool.tile([BS, K], f32)
            nc.sync.dma_start(out=io[:], in_=iota[:])
            statefin = cpool.tile([BS, K], f32)
            bps = bpool.tile([BS, (T - 1) * K], i16)

            tr3d = tr[:].rearrange("p (j i) -> p j i", j=K)

            # ---------------- forward ----------------
            with (
                tc.tile_pool(name="pot", bufs=2) as ppool,
                tc.tile_pool(name="fwd", bufs=2) as fpool,
                tc.tile_pool(name="st", bufs=3) as spool,
            ):
                state = None
                for c in range(T // CHUNK):
                    ptile = ppool.tile([BS, CHUNK, K], f32, tag="pot")
                    nc.sync.dma_start(
                        out=ptile[:], in_=pot[:, c * CHUNK : (c + 1) * CHUNK, :]
                    )
                    for s in range(CHUNK):
                        t = c * CHUNK + s
                        if t == 0:
                            state = spool.tile([BS, K], f32, tag="state")
                            nc.vector.tensor_copy(state[:], ptile[:, 0, :])
                            continue
                        v = fpool.tile([BS, K * K], f32, tag="v")
                        v3d = v[:].rearrange("p (j i) -> p j i", j=K)
                        sbc = (
                            state[:]
                            .rearrange("p (a i) -> p a i", a=1)
                            .broadcast_to([BS, K, K])
                        )
                        nc.vector.tensor_tensor(out=v3d, in0=sbc, in1=tr3d, op=ADD)
                        maxs = fpool.tile([BS, K], f32, tag="maxs")
                        nc.vector.tensor_reduce(out=maxs[:], in_=v3d, axis=X, op=MAX)
                        newstate = spool.tile([BS, K], f32, tag="state")
                        nc.gpsimd.tensor_tensor(
                            out=newstate[:], in0=maxs[:], in1=ptile[:, s, :], op=ADD
                        )
                        sel = fpool.tile([BS, K * K], f32, tag="sel")
                        sel3d = sel[:].rearrange("p (j i) -> p j i", j=K)
                        maxsBC = (
                            maxs[:]
                            .rearrange("p (j a) -> p j a", a=1)
                            .broadcast_to([BS, K, K])
                        )
                        nc.vector._custom_dve(
                            OP, out=sel3d, in0=v3d, in1=maxsBC, s0=float(K * K)
                        )
                        bslot = bps[:, (t - 1) * K : t * K]
                        nc.vector.tensor_reduce(out=bslot, in_=sel3d, axis=X, op=MAX)
                        state = newstate
                    del ptile
                nc.vector.tensor_copy(statefin[:], state[:])

            # ---------------- backward ----------------
            with (
                tc.tile_pool(name="outp", bufs=2) as opool,
                tc.tile_pool(name="bwd", bufs=4) as wpool,
            ):
                # final tag: first-index argmax of statefin via the same encoding
                mfin = wpool.tile([BS, 1], f32, tag="wv")
                nc.vector.tensor_reduce(out=mfin[:], in_=statefin[:], axis=X, op=MAX)
                self_ = wpool.tile([BS, K], f32, tag="prod")
                nc.vector._custom_dve(
                    OP,
                    out=self_[:].rearrange("p (a i) -> p a i", a=1),
                    in0=statefin[:].rearrange("p (a i) -> p a i", a=1),
                    in1=mfin[:]
                    .rearrange("p (a i) -> p a i", a=1)
                    .broadcast_to([BS, 1, K]),
                    s0=float(K),
                )
                rfin = wpool.tile([BS, 1], f32, tag="wv")
                nc.vector.tensor_reduce(out=rfin[:], in_=self_[:], axis=X, op=MAX)
                ochunk = opool.tile([BS, CHUNK, K], f32, tag="out")
                # h[k] = (k + r == K)  <=>  k = K - r = argmax
                nc.vector.tensor_scalar(
                    out=ochunk[:, CHUNK - 1, :], in0=io[:], scalar1=rfin[:],
                    scalar2=float(K), op0=ADD, op1=EQ,
                )
                x = wpool.tile([BS, 1], f32, tag="x")
                # x = K - r
                nc.vector.tensor_scalar(
                    out=x[:], in0=rfin[:], scalar1=-1.0, scalar2=float(K),
                    op0=MUL, op1=ADD,
                )
                hprev = ochunk[:, CHUNK - 1, :]
                for t in range(T - 2, -1, -1):
                    s = t % CHUNK
                    prod = wpool.tile([BS, K], f32, tag="prod")
                    nc.vector.tensor_tensor(
                        out=prod[:], in0=bps[:, t * K : (t + 1) * K], in1=hprev, op=MUL
                    )
                    wv = wpool.tile([BS, 1], f32, tag="wv")
                    nc.vector.tensor_reduce(out=wv[:], in_=prod[:], axis=X, op=MAX)
                    # e = 1024 - 32 * x_{t+1}
                    e = wpool.tile([BS, 1], f32, tag="e")
                    nc.vector.tensor_scalar(
                        out=e[:], in0=x[:], scalar1=-float(K), scalar2=float(K * K),
                        op0=MUL, op1=ADD,
                    )
                    # h_t[k] = (k + wv == e)
                    nc.vector.tensor_scalar(
                        out=ochunk[:, s, :], in0=io[:], scalar1=wv[:], scalar2=e[:],
                        op0=ADD, op1=EQ,
                    )
                    # x_t = e - wv
                    x = wpool.tile([BS, 1], f32, tag="x")
                    nc.vector.tensor_scalar(
                        out=x[:], in0=e[:], scalar1=wv[:], scalar2=None, op0=SUB
                    )
                    hprev = ochunk[:, s, :]
                    if s == 0:
                        c = t // CHUNK
                        nc.sync.dma_start(
                            out=out[:, c * CHUNK : (c + 1) * CHUNK, :], in_=ochunk[:]
                        )
                        if t > 0:
                            ochunk = opool.tile([BS, CHUNK, K], f32, tag="out")

    mybir.codegen_inst_isa_subclasses(nc)
    _split_waits(nc, mybir)
    return nc


def _get_program():
    if "nc" not in _cached:
        _cached["nc"] = _build()
    return _cached["nc"]


def _ensure_profile_hook():
    import sys
    import types

    try:
        from antenv.axon_hooks import get_axon_ntff_profile_hook  # noqa: F401
        return
    except ImportError:
        pass
    import antenv
    from trn_agent_boot.trn_boot import _ntff_profile_via_ctypes

    m = types.ModuleType("antenv.axon_hooks")
    holder = {"h": None}
    m.set_axon_ntff_profile_hook = lambda h: holder.__setitem__("h", h)
    m.get_axon_ntff_profile_hook = lambda: holder["h"]
    sys.modules["antenv.axon_hooks"] = m
    antenv.axon_hooks = m
    m.set_axon_ntff_profile_hook(
        _ntff_profile_via_ctypes("/opt/axon/libaxon_pjrt.so")
    )


def _run(inputs, transitions, trace=False):
    from concourse.bass_utils import run_bass_kernel_spmd

    if trace:
        _ensure_profile_hook()
    nc = _get_program()
    transrep = np.broadcast_to(
        np.ascontiguousarray(transitions.T).reshape(1, K * K), (BS, K * K)
    ).copy()
    iota = np.broadcast_to(
        np.arange(K, dtype=np.float32).reshape(1, K), (BS, K)
    ).copy()
    in_maps = [
        {
            "pot": np.ascontiguousarray(inputs[c * BS : (c + 1) * BS]),
            "transrep": transrep,
            "iota": iota,
        }
        for c in range(NCORES)
    ]
    res = run_bass_kernel_spmd(nc, in_maps, list(range(NCORES)), trace=trace)
    full = np.concatenate([res.results[c]["out"] for c in range(NCORES)], axis=0)
    return full, res


def kernel(inputs, transitions):
    inputs = np.asarray(inputs, dtype=np.float32)
    transitions = np.asarray(transitions, dtype=np.float32)
    full, _ = _run(inputs, transitions, trace=False)
    return full.astype(np.float32)


# revision 4
# speedup vs baseline: 1.6101x; 1.4029x over previous
"""Viterbi CRF decode kernel for Trainium2 (8 NeuronCores, SPMD data-parallel).

Problem: inputs [512, 2048, 32] f32 potentials, transitions [32, 32] f32.
Output: one_hot(viterbi_tags) [512, 2048, 32] f32.

Sharding: batch 512 -> 8 cores x 64. Per core, the forward Viterbi scan runs
in a split layout using all 128 partitions: partition p = jh*64 + b computes
the j-half jh (16 of the 32 next-tags) for batch b, halving every DVE stream:

  v[p, jg, i]  = state[b, i] + T[i, 16*jh+jg]    (broadcast TT add, 512 elems)
  maxs[p, jg]  = max_i v                         (strided X-reduce)
  w[p, jg]     = max_i ((v == maxs) ? (1024 - 512*jh) - Idx : 0)  -> int16
                 (fused custom DVE op; decreasing-index encoding makes ties
                  resolve to the FIRST i like jnp.argmax, and the per-partition
                  s0 column restores the global encoding w = 1024 - 32j - i)
  full state rebuild across partition halves via two tiny PE permutation
  matmuls (bitwise exact) + one small add of the potentials:
  state'[p, j] = (Perm @ maxs)[p, j] + pot[b, t, j]

Backpointer words w spill to a DRAM scratch (repacked to [b, t, 32] by DMA);
the backward pass streams them back and emits one-hot rows directly into the
output buffer:  wv = max_j(w_t[j] * h_{t+1}[j]);  e = 1024 - 32*x_{t+1};
h_t[k] = (k + wv == e);  x_t = e - wv.   All arithmetic is exact in fp32, so
the result matches the jax reference bit-for-bit.
"""

import numpy as np

B, T, K = 512, 2048, 32
NCORES = 8
BS = B // NCORES  # 64 batches per core
KH = K // 2       # 16, one j-half per partition group
CHUNK = 128       # time chunk for potentials / bps / output DMA

_cached = {}

_OP_SHAS = {"v3": "130386e9d5d4f156", "v4": "874d42a2619fbd85"}


def _register_custom_op():
    import concourse.dve_ops as dve_ops
    from concourse.dve_ops import DveOp, OPS
    from concourse.dve_spec import Spec, Src0, Src1, C0, Idx, Zero, select, eq

    for o in OPS:
        if o.name == "CRF_SELECT_EQ_IDX":
            return o

    def _ref(in0, in1, s0, s1, imm2):
        n = in0.shape[-1] if in0.ndim == 2 else in0.shape[-1] * in0.shape[-2]
        idx = np.arange(n, dtype=np.float32).reshape(in0.shape[1:])
        return np.where(in0 == in1, s0 - idx[None], 0.0)

    spec = Spec(body=select(eq(Src0, Src1), C0 - Idx, Zero), reference=_ref)
    op = DveOp("CRF_SELECT_EQ_IDX", spec, subdim=False, uops_sha=dict(_OP_SHAS))
    OPS.append(op)
    dve_ops._SUB_OPCODE_FOR_NAME[op.name] = (
        dve_ops._CUSTOM_DVE_ROW_BASE + len(OPS) - 1
    )
    return op


def _split_waits(nc, mybir):
    """This walrus build accepts at most one sync-wait per instruction; move
    extra waits onto preceding EventSemaphore instructions on the same engine."""
    for f in nc.m.functions:
        for blk in f.blocks:
            new_list = []
            for ins in blk.instructions:
                si = ins.sync_info
                if si is not None and len(si.on_wait) > 1:
                    waits = list(si.on_wait)
                    keep, moved = waits[-1:], waits[:-1]
                    for ci, w in enumerate(moved):
                        nop = mybir.InstEventSemaphore(name=f"{ins.name}-ws{ci}")
                        nop.engine = ins.engine
                        nop.sync_info = mybir.SyncInfo(on_wait=[w], on_update=[])
                        new_list.append(nop)
                    ins.sync_info = mybir.SyncInfo(
                        on_wait=keep, on_update=list(si.on_update)
                    )
                new_list.append(ins)
            blk.instructions = new_list


def _build():
    import concourse.bass as bass
    import concourse.mybir as mybir
    from concourse.tile import TileContext

    OP = _register_custom_op()

    f32 = mybir.dt.float32
    i16 = mybir.dt.int16
    ADD = mybir.AluOpType.add
    SUB = mybir.AluOpType.subtract
    MUL = mybir.AluOpType.mult
    MAX = mybir.AluOpType.max
    EQ = mybir.AluOpType.is_equal
    X = mybir.AxisListType.X

    nc = bass.Bass("TRN2", target_bir_lowering=False, num_devices=NCORES)
    pot = nc.declare_dram_parameter("pot", [BS, T, K], f32, isOutput=False)
    transrep = nc.declare_dram_parameter("transrep", [2 * BS, KH * K], f32,
                                         isOutput=False)
    iota = nc.declare_dram_parameter("iota", [BS, K], f32, isOutput=False)
    colc = nc.declare_dram_parameter("colc", [2 * BS, 1], f32, isOutput=False)
    perma = nc.declare_dram_parameter("perma", [2 * BS, 2 * BS], f32,
                                      isOutput=False)
    permb = nc.declare_dram_parameter("permb", [2 * BS, 2 * BS], f32,
                                      isOutput=False)
    out = nc.declare_dram_parameter("out", [BS, T, K], f32, isOutput=True)
    bscratch = nc.dram_tensor("bscratch", [BS, T * K], i16)
    bs3 = bscratch[:].rearrange("b (t k) -> b t k", t=T)

    P = 2 * BS  # 128

    with TileContext(nc) as tc:
        with tc.tile_pool(name="const", bufs=1) as cpool:
            tr = cpool.tile([P, KH * K], f32)
            nc.sync.dma_start(out=tr[:], in_=transrep[:])
            io = cpool.tile([BS, K], f32)
            nc.sync.dma_start(out=io[:], in_=iota[:])
            cc = cpool.tile([P, 1], f32)
            nc.sync.dma_start(out=cc[:], in_=colc[:])
            pa = cpool.tile([P, P], f32)
            nc.sync.dma_start(out=pa[:], in_=perma[:])
            pb = cpool.tile([P, P], f32)
            nc.sync.dma_start(out=pb[:], in_=permb[:])
            statefin = cpool.tile([BS, K], f32)

            tr3d = tr[:].rearrange("p (j i) -> p j i", j=KH)

            # ---------------- forward ----------------
            with (
                tc.tile_pool(name="pot", bufs=2) as ppool,
                tc.tile_pool(name="fwd", bufs=2) as fpool,
                tc.tile_pool(name="st", bufs=3) as spool,
                tc.tile_pool(name="bch", bufs=2) as bcpool,
                tc.tile_pool(name="ps", bufs=2, space="PSUM") as pspool,
            ):
                statefull = None
                for c in range(T // CHUNK):
                    ptile = ppool.tile([P, CHUNK, K], f32, tag="pot")
                    nc.sync.dma_start(
                        out=ptile[0:BS], in_=pot[:, c * CHUNK : (c + 1) * CHUNK, :]
                    )
                    nc.sync.dma_start(
                        out=ptile[BS:P], in_=pot[:, c * CHUNK : (c + 1) * CHUNK, :]
                    )
                    bchunk = bcpool.tile([P, CHUNK * KH], i16, tag="bch")
                    for s in range(CHUNK):
                        t = c * CHUNK + s
                        if t == 0:
                            statefull = spool.tile([P, K], f32, tag="state")
                            nc.vector.tensor_copy(statefull[:], ptile[:, 0, :])
                            nc.vector.memset(bchunk[:, 0:KH], 0)
                            continue
                        v = fpool.tile([P, KH * K], f32, tag="v")
                        v3d = v[:].rearrange("p (j i) -> p j i", j=KH)
                        sbc = (
                            statefull[:]
                            .rearrange("p (a i) -> p a i", a=1)
                            .broadcast_to([P, KH, K])
                        )
                        nc.vector.tensor_tensor(out=v3d, in0=sbc, in1=tr3d, op=ADD)
                        maxs = fpool.tile([P, KH], f32, tag="maxs")
                        nc.vector.tensor_reduce(out=maxs[:], in_=v3d, axis=X, op=MAX)
                        sel = fpool.tile([P, KH * K], f32, tag="sel")
                        sel3d = sel[:].rearrange("p (j i) -> p j i", j=KH)
                        maxsBC = (
                            maxs[:]
                            .rearrange("p (j a) -> p j a", a=1)
                            .broadcast_to([P, KH, K])
                        )
                        nc.vector._custom_dve(
                            OP, out=sel3d, in0=v3d, in1=maxsBC, s0=cc[:]
                        )
                        bslot = bchunk[:, s * KH : (s + 1) * KH]
                        nc.vector.tensor_reduce(out=bslot, in_=sel3d, axis=X, op=MAX)
                        # rebuild full state on every partition: two exact
                        # permutation matmuls gather both j-halves, then add pot
                        sfps = pspool.tile([P, K], f32, tag="sf")
                        nc.tensor.matmul(sfps[:, 0:KH], pa[:], maxs[:])
                        nc.tensor.matmul(sfps[:, KH:K], pb[:], maxs[:])
                        statefull = spool.tile([P, K], f32, tag="state")
                        nc.vector.tensor_tensor(
                            out=statefull[:], in0=sfps[:], in1=ptile[:, s, :], op=ADD
                        )
                    # spill this chunk's backpointer words to DRAM, repacked
                    bc3 = bchunk[:].rearrange("p (t j) -> p t j", t=CHUNK)
                    dst = bs3[:, c * CHUNK : (c + 1) * CHUNK, :]
                    nc.sync.dma_start(out=dst[:, :, 0:KH], in_=bc3[0:BS])
                    nc.sync.dma_start(out=dst[:, :, KH:K], in_=bc3[BS:P])
                    del ptile, bchunk
                nc.vector.tensor_copy(statefin[:], statefull[0:BS, :])

            # ---------------- backward ----------------
            with (
                tc.tile_pool(name="outp", bufs=2) as opool,
                tc.tile_pool(name="bwd", bufs=4) as wpool,
                tc.tile_pool(name="bin", bufs=2) as bipool,
            ):
                # final tag: first-index argmax of statefin via the same encoding
                mfin = wpool.tile([BS, 1], f32, tag="wv")
                nc.vector.tensor_reduce(out=mfin[:], in_=statefin[:], axis=X, op=MAX)
                self_ = wpool.tile([BS, K], f32, tag="prod")
                nc.vector._custom_dve(
                    OP,
                    out=self_[:].rearrange("p (a i) -> p a i", a=1),
                    in0=statefin[:].rearrange("p (a i) -> p a i", a=1),
                    in1=mfin[:]
                    .rearrange("p (a i) -> p a i", a=1)
                    .broadcast_to([BS, 1, K]),
                    s0=float(K),
                )
                rfin = wpool.tile([BS, 1], f32, tag="wv")
                nc.vector.tensor_reduce(out=rfin[:], in_=self_[:], axis=X, op=MAX)
                ochunk = opool.tile([BS, CHUNK, K], f32, tag="out")
                # h[k] = (k + r == K)  <=>  k = K - r = argmax
                nc.vector.tensor_scalar(
                    out=ochunk[:, CHUNK - 1, :], in0=io[:], scalar1=rfin[:],
                    scalar2=float(K), op0=ADD, op1=EQ,
                )
                x = wpool.tile([BS, 1], f32, tag="x")
                nc.vector.tensor_scalar(
                    out=x[:], in0=rfin[:], scalar1=-1.0, scalar2=float(K),
                    op0=MUL, op1=ADD,
                )
                hprev = ochunk[:, CHUNK - 1, :]
                btile_next = None
                btile_cur = bipool.tile([BS, CHUNK, K], i16, tag="bin")
                lastc = T // CHUNK - 1
                nc.sync.dma_start(
                    out=btile_cur[:],
                    in_=bs3[:, lastc * CHUNK : (lastc + 1) * CHUNK, :],
                )
                for t in range(T - 2, -1, -1):
                    s = t % CHUNK
                    # w for step t+1 lives at bscratch slot t+1
                    if s == CHUNK - 1:
                        wslot = btile_next[:, 0, :]
                    else:
                        wslot = btile_cur[:, s + 1, :]
                    prod = wpool.tile([BS, K], f32, tag="prod")
                    nc.vector.tensor_tensor(out=prod[:], in0=wslot, in1=hprev, op=MUL)
                    wv = wpool.tile([BS, 1], f32, tag="wv")
                    nc.vector.tensor_reduce(out=wv[:], in_=prod[:], axis=X, op=MAX)
                    # e = 1024 - 32 * x_{t+1}
                    e = wpool.tile([BS, 1], f32, tag="e")
                    nc.vector.tensor_scalar(
                        out=e[:], in0=x[:], scalar1=-float(K), scalar2=float(K * K),
                        op0=MUL, op1=ADD,
                    )
                    # h_t[k] = (k + wv == e)
                    nc.vector.tensor_scalar(
                        out=ochunk[:, s, :], in0=io[:], scalar1=wv[:], scalar2=e[:],
                        op0=ADD, op1=EQ,
                    )
                    # x_t = e - wv
                    x = wpool.tile([BS, 1], f32, tag="x")
                    nc.vector.tensor_scalar(
                        out=x[:], in0=e[:], scalar1=wv[:], scalar2=None, op0=SUB
                    )
                    hprev = ochunk[:, s, :]
                    if s == 0:
                        c = t // CHUNK
                        nc.sync.dma_start(
                            out=out[:, c * CHUNK : (c + 1) * CHUNK, :], in_=ochunk[:]
                        )
                        if t > 0:
                            ochunk = opool.tile([BS, CHUNK, K], f32, tag="out")
                            btile_next = btile_cur
                            btile_cur = bipool.tile([BS, CHUNK, K], i16, tag="bin")
                            nc.sync.dma_start(
                                out=btile_cur[:],
                                in_=bs3[:, (c - 1) * CHUNK : c * CHUNK, :],
                            )

    mybir.codegen_inst_isa_subclasses(nc)
    _split_waits(nc, mybir)
    return nc


def _get_program():
    if "nc" not in _cached:
        _cached["nc"] = _build()
    return _cached["nc"]


def _ensure_profile_hook():
    import sys
    import types

    try:
        from antenv.axon_hooks import get_axon_ntff_profile_hook  # noqa: F401
        return
    except ImportError:
        pass
    import antenv
    from trn_agent_boot.trn_boot import _ntff_profile_via_ctypes

    m = types.ModuleType("antenv.axon_hooks")
    holder = {"h": None}
    m.set_axon_ntff_profile_hook = lambda h: holder.__setitem__("h", h)
    m.get_axon_ntff_profile_hook = lambda: holder["h"]
    sys.modules["antenv.axon_hooks"] = m
    antenv.axon_hooks = m
    m.set_axon_ntff_profile_hook(
        _ntff_profile_via_ctypes("/opt/axon/libaxon_pjrt.so")
    )


def _host_constants(transitions):
    # transrep_split[p=(jh,b), 32*jg + i] = T[i, 16*jh + jg]
    TT = np.ascontiguousarray(transitions.T)  # [j, i]
    half0 = TT[0:KH, :].reshape(1, KH * K)
    half1 = TT[KH:K, :].reshape(1, KH * K)
    transrep = np.concatenate(
        [np.broadcast_to(half0, (BS, KH * K)), np.broadcast_to(half1, (BS, KH * K))],
        axis=0,
    ).astype(np.float32)
    iota = np.broadcast_to(
        np.arange(K, dtype=np.float32).reshape(1, K), (BS, K)
    ).copy()
    colc = np.concatenate(
        [np.full((BS, 1), float(K * K), np.float32),
         np.full((BS, 1), float(K * K - KH * K), np.float32)],
        axis=0,
    )
    PA = np.zeros((2 * BS, 2 * BS), np.float32)
    PB = np.zeros((2 * BS, 2 * BS), np.float32)
    for p in range(2 * BS):
        PA[p % BS, p] = 1.0
        PB[BS + (p % BS), p] = 1.0
    return transrep, iota, colc, PA, PB


def _run(inputs, transitions, trace=False):
    from concourse.bass_utils import run_bass_kernel_spmd

    if trace:
        _ensure_profile_hook()
    nc = _get_program()
    transrep, iota, colc, PA, PB = _host_constants(transitions)
    in_maps = [
        {
            "pot": np.ascontiguousarray(inputs[c * BS : (c + 1) * BS]),
            "transrep": transrep,
            "iota": iota,
            "colc": colc,
            "perma": PA,
            "permb": PB,
        }
        for c in range(NCORES)
    ]
    res = run_bass_kernel_spmd(nc, in_maps, list(range(NCORES)), trace=trace)
    full = np.concatenate([res.results[c]["out"] for c in range(NCORES)], axis=0)
    return full, res


def kernel(inputs, transitions):
    inputs = np.asarray(inputs, dtype=np.float32)
    transitions = np.asarray(transitions, dtype=np.float32)
    full, _ = _run(inputs, transitions, trace=False)
    return full.astype(np.float32)
